# revision 15
# baseline (speedup 1.0000x reference)
"""Expert-parallel MoE MLP (ExpertMLP) Bass kernel for 8 Trainium2 NeuronCores.

Problem: x[32,4096,256] @ w_fc[32,256,1024] -> gelu(erf) -> @ w_proj[32,1024,256].

Sharding: expert-parallel. Each of the 8 cores gets 4 experts (slices of the
leading axis of every tensor); no cross-core communication. Inside a core, per
expert e:

  1. xT [d, c] in bf16 is produced fully on-chip: each 1024-row slab of
     x[e] is DMA'd f32 DRAM->SBUF in natural [cap, d] orientation (contiguous,
     fast), DVE-cast to bf16 (round-to-nearest), then XBar DMA-transposed
     SBUF->SBUF in [128,128] blocks into xT tiles. This keeps every
     dependency SBUF-tile-granular - no DRAM->DRAM cast pass (which
     serialized ~60us of queue drain ahead of the first matmul in earlier
     versions) and no software-DGE involvement.
  2. MM1: hT[h_tile, c_chunk] += w_fc_tile.T @ xT_chunk - w_fc's natural
     [d, h] layout is the stationary operand (DVE-cast to bf16 on load).
  3. GELU (exact erf form) runs on the ACT engine as the PSUM->SBUF eviction.
  4. MM2 runs k-major over all capacity subtiles of the chunk: for k, for s:
     out[s] += hT[k][s].T @ w_proj[k]. The first MM2 instructions only need
     the chunk's first GELU group, so the PE never waits for the last GELU.
     Each subtile accumulator owns a FULL 2KB PSUM bank (start=True clears
     the whole bank). Results land directly in [capacity, d] orientation.

xT staging for expert e+1 is issued interleaved with expert e's chunk loop
(half a slab per chunk) so the SP queue serves output DMAs promptly and xT
production stays ~1 expert ahead of consumption.

All matmul operands are bf16; PSUM accumulation stays fp32.
"""

import numpy as np
from contextlib import ExitStack

import bass_rust as _br
import concourse.bass as bass
import concourse.tile as tile
from concourse import mybir
from concourse.bass_utils import run_bass_kernel_spmd

E, CAP, D, H = 32, 4096, 256, 1024
N_CORES = 8
E_PER = E // N_CORES  # 4 experts per core
P = 128
F32 = mybir.dt.float32
BF16 = mybir.dt.bfloat16

KD = D // P        # 2 k-tiles in MM1's contraction
KH = H // P        # 8 k-tiles in MM2's contraction
NC_CHUNK = 512     # capacity chunk processed per MM1/MM2 round
N_CHUNKS = CAP // NC_CHUNK
H_TILES = H // P
SLAB = 1024        # xT staging slab (capacity rows)
N_SLABS = CAP // SLAB
Q = SLAB // P      # 8 capacity subtiles per slab


def _fix_waits(nc):
    """walrus accepts only one sync wait per instruction (and none at all on
    gpsimd DIRECT2D DMAs); hoist excess waits onto standalone EventSemaphore
    instructions inserted before the offender (same engine => same sequencer
    order)."""
    for fn in nc.m.functions:
        for bb in fn.blocks:
            new = []
            changed = False
            for inst in bb.instructions:
                si = inst.sync_info
                keep = 0 if (
                    si is not None
                    and inst.engine == mybir.EngineType.Pool
                    and type(inst).__name__ in ("InstDMACopy", "InstTensorCopy")
                    and len(si.on_wait) >= 1
                ) else 1
                if si is not None and len(si.on_wait) > keep:
                    waits = list(si.on_wait)
                    for w in waits[: len(waits) - keep]:
                        ev = mybir.InstEventSemaphore(
                            name=nc.get_next_instruction_name()
                        )
                        ev.engine = inst.engine
                        ev.sync_info = _br.SyncInfo(on_wait=[w], on_update=[])
                        nc.register_instruction(ev)
                        new.append(ev)
                    inst.sync_info = _br.SyncInfo(
                        on_wait=waits[len(waits) - keep:],
                        on_update=list(si.on_update),
                    )
                    changed = True
                new.append(inst)
            if changed:
                bb.instructions = new


def _build():
    nc = bass.Bass(trn_type="TRN2", target_bir_lowering=False, debug=False)
    x = nc.dram_tensor("x", [E_PER, CAP, D], F32, kind="ExternalInput").ap()
    w_fc = nc.dram_tensor("w_fc", [E_PER, D, H], F32, kind="ExternalInput").ap()
    w_proj = nc.dram_tensor("w_proj", [E_PER, H, D], F32, kind="ExternalInput").ap()
    out = nc.dram_tensor("out", [E_PER, CAP, D], F32, kind="ExternalOutput").ap()

    with tile.TileContext(nc) as tc, ExitStack() as ctx:
        xrp = ctx.enter_context(tc.tile_pool(name="xrp", bufs=2))
        xbp = ctx.enter_context(tc.tile_pool(name="xbp", bufs=2))
        xtp = ctx.enter_context(tc.tile_pool(name="xtp", bufs=2 * N_SLABS * KD))
        wload = ctx.enter_context(tc.tile_pool(name="wload", bufs=2))
        wfc_p = ctx.enter_context(tc.tile_pool(name="wfc", bufs=2))
        wproj_p = ctx.enter_context(tc.tile_pool(name="wproj", bufs=2))
        ht_p = ctx.enter_context(tc.tile_pool(name="ht", bufs=8))
        out_p = ctx.enter_context(tc.tile_pool(name="outp", bufs=3))
        ps_h = ctx.enter_context(tc.tile_pool(name="ps_h", bufs=2, space="PSUM"))
        ps_o = ctx.enter_context(tc.tile_pool(name="ps_o", bufs=4, space="PSUM"))

        HPACK = 2          # h_tiles packed per PSUM tile / GELU call

        def load_weights(e):
            wfc_raw = wload.tile([P, KD, H], F32, tag="wl")
            nc.sync.dma_start(wfc_raw[:], w_fc[e].rearrange("(k p) h -> p k h", p=P))
            wfc = wfc_p.tile([P, KD, H], BF16, tag="wfc")
            nc.vector.tensor_copy(wfc[:], wfc_raw[:])
            wproj_raw = wload.tile([P, KH, D], F32, tag="wl")
            nc.sync.dma_start(
                wproj_raw[:], w_proj[e].rearrange("(k p) d -> p k d", p=P)
            )
            wproj = wproj_p.tile([P, KH, D], BF16, tag="wproj")
            nc.vector.tensor_copy(wproj[:], wproj_raw[:])
            return wfc, wproj

        def make_xt(e):
            return [
                [
                    xtp.tile([P, SLAB], BF16, tag="xt", name=f"xt{e}_{k}_{s}")
                    for s in range(N_SLABS)
                ]
                for k in range(KD)
            ]

        xbfs = {}

        def stage_slab(e, s):
            """Load slab s of x[e] into SBUF (f32) and DVE-cast to bf16."""
            cs = slice(s * SLAB, (s + 1) * SLAB)
            xraw = xrp.tile([P, Q, D], F32, tag="xr", name=f"xr{e}_{s}")
            nc.sync.dma_start(
                xraw[:], x[e][cs].rearrange("(q p) d -> p q d", p=P)
            )
            xb = xbp.tile([P, Q, D], BF16, tag="xb", name=f"xb{e}_{s}")
            nc.vector.tensor_copy(xb[:], xraw[:])
            xbfs[(e, s)] = xb

        def issue_transposes(e, xt, s, qhalf):
            """XBar-transpose [128,128] blocks of slab s (q-range half) into
            the xT tiles."""
            xb = xbfs[(e, s)]
            for q in range(4 * qhalf, 4 * qhalf + 4):
                for k in range(KD):
                    nc.sync.dma_start_transpose(
                        xt[k][s][:, q * P:(q + 1) * P],
                        xb[:, q, k * P:(k + 1) * P],
                    )
            if qhalf == 1:
                del xbfs[(e, s)]

        # ---- prologue: expert 0's weights + full xT ----
        w = [None] * E_PER
        xts = [None] * E_PER
        w[0] = load_weights(0)
        xts[0] = make_xt(0)
        for s in range(N_SLABS):
            stage_slab(0, s)
            issue_transposes(0, xts[0], s, 0)
            issue_transposes(0, xts[0], s, 1)

        for e in range(E_PER):
            xt = xts[e]
            wfc, wproj = w[e]

            for nci in range(N_CHUNKS):
                # stage expert e+1's xT half a slab per chunk, weights at
                # the first chunk.
                if e + 1 < E_PER:
                    if nci == 0:
                        w[e + 1] = load_weights(e + 1)
                        xts[e + 1] = make_xt(e + 1)
                    if nci % 2 == 0:
                        stage_slab(e + 1, nci // 2)
                    issue_transposes(e + 1, xts[e + 1], nci // 2, nci % 2)

                csl = slice(nci * NC_CHUNK, (nci + 1) * NC_CHUNK)
                sidx = (nci * NC_CHUNK) // SLAB
                soff = (nci * NC_CHUNK) % SLAB
                # ---- MM1 + GELU: HPACK h_tiles per PSUM tile / GELU call ----
                ht_tiles = []
                for hp in range(H_TILES // HPACK):
                    psh = ps_h.tile([P, HPACK, NC_CHUNK], F32, tag="psh")
                    for j in range(HPACK):
                        hi = hp * HPACK + j
                        for k in range(KD):
                            nc.tensor.matmul(
                                psh[:, j, :],
                                wfc[:, k, hi * P:(hi + 1) * P],
                                xt[k][sidx][:, soff:soff + NC_CHUNK],
                                start=(k == 0),
                                stop=(k == KD - 1),
                            )
                    ht = ht_p.tile([P, HPACK, NC_CHUNK], BF16, tag="ht")
                    nc.scalar.activation(
                        ht[:], psh[:], mybir.ActivationFunctionType.Gelu
                    )
                    ht_tiles.append(ht)

                # ---- MM2, k-major: psum[s] += hT[k][s].T @ w_proj[k] ----
                NS = NC_CHUNK // P  # 4 capacity subtiles
                psos = [
                    ps_o.tile([P, 2 * D], F32, tag="pso", name=f"pso{e}_{nci}_{i}")
                    for i in range(NS)
                ]
                for k in range(KH):
                    for s in range(NS):
                        nc.tensor.matmul(
                            psos[s][:, :D],
                            ht_tiles[k // HPACK][:, k % HPACK, s * P:(s + 1) * P],
                            wproj[:, k, :],
                            start=(k == 0),
                            stop=(k == KH - 1),
                        )
                ob = out_p.tile([P, NC_CHUNK // P, D], F32, tag="ob")
                for s, pso in enumerate(psos):
                    nc.vector.tensor_copy(ob[:, s, :], pso[:, :D])
                nc.sync.dma_start(
                    out[e, csl, :].rearrange("(s p) d -> p s d", p=P), ob[:]
                )

    _fix_waits(nc)
    return nc


_CACHE = {}


def _get_nc():
    if "nc" not in _CACHE:
        _CACHE["nc"] = _build()
    return _CACHE["nc"]


def kernel(x, w_fc, w_proj, trace=False):
    assert x.shape == (E, CAP, D) and w_fc.shape == (E, D, H)
    assert w_proj.shape == (E, H, D)
    nc = _get_nc()
    x = np.ascontiguousarray(x, dtype=np.float32)
    w_fc = np.ascontiguousarray(w_fc, dtype=np.float32)
    w_proj = np.ascontiguousarray(w_proj, dtype=np.float32)
    in_maps = [
        {
            "x": x[i * E_PER:(i + 1) * E_PER],
            "w_fc": w_fc[i * E_PER:(i + 1) * E_PER],
            "w_proj": w_proj[i * E_PER:(i + 1) * E_PER],
        }
        for i in range(N_CORES)
    ]
    res = run_bass_kernel_spmd(nc, in_maps, list(range(N_CORES)), trace=trace)
    out = np.concatenate([r["out"] for r in res.results], axis=0)
    if trace:
        kernel.last_results = res
    return out


# revision 16
# speedup vs baseline: 1.6947x; 1.6947x over previous
"""Expert-parallel MoE MLP (ExpertMLP) Bass kernel for 8 Trainium2 NeuronCores.

Problem: x[32,4096,256] @ w_fc[32,256,1024] -> gelu(erf) -> @ w_proj[32,1024,256].

Sharding: expert-parallel. Each of the 8 cores gets 4 experts (slices of the
leading axis of every tensor); no cross-core communication. Inside a core, per
expert e:

  1. xT [d, c] in bf16: each 1024-row slab of x[e] is DMA'd f32 DRAM->SBUF
     in natural [cap, d] orientation (contiguous, fast), DVE-cast to bf16
     (round-to-nearest), written back to a per-slab DRAM staging tensor, and
     XBar DMA-transposed DRAM->SBUF as two [1024,128] blocks. All four steps
     are HWDGE/DVE ops with per-slab dependencies, so the pipeline starts
     ~10us in - unlike the software-DGE DRAM->DRAM cast pass (55us of queue
     drain before the first matmul) or per-[128,128]-block SBUF XBar calls
     (a DMA_TRANSPOSE costs ~1.2us of engine time regardless of size).
  2. MM1: hT[h_tile, c_chunk] += w_fc_tile.T @ xT_chunk - w_fc's natural
     [d, h] layout is the stationary operand (DVE-cast to bf16 on load).
  3. GELU (exact erf form) runs on the ACT engine as the PSUM->SBUF eviction.
  4. MM2 runs k-major over all capacity subtiles of the chunk: for k, for s:
     out[s] += hT[k][s].T @ w_proj[k]. The first MM2 instructions only need
     the chunk's first GELU group, so the PE never waits for the last GELU.
     Each subtile accumulator owns a FULL 2KB PSUM bank (start=True clears
     the whole bank). Results land directly in [capacity, d] orientation.

xT staging for expert e+1 is issued interleaved with expert e's chunk loop
(half a slab per chunk) so the SP queue serves output DMAs promptly and xT
production stays ~1 expert ahead of consumption.

All matmul operands are bf16; PSUM accumulation stays fp32.
"""

import numpy as np
from contextlib import ExitStack

import bass_rust as _br
import concourse.bass as bass
import concourse.tile as tile
from concourse import mybir
from concourse.bass_utils import run_bass_kernel_spmd

E, CAP, D, H = 32, 4096, 256, 1024
N_CORES = 8
E_PER = E // N_CORES  # 4 experts per core
P = 128
F32 = mybir.dt.float32
BF16 = mybir.dt.bfloat16

KD = D // P        # 2 k-tiles in MM1's contraction
KH = H // P        # 8 k-tiles in MM2's contraction
NC_CHUNK = 512     # capacity chunk processed per MM1/MM2 round
N_CHUNKS = CAP // NC_CHUNK
H_TILES = H // P
SLAB = 1024        # xT staging slab (capacity rows)
N_SLABS = CAP // SLAB
Q = SLAB // P      # 8 capacity subtiles per slab


def _fix_waits(nc):
    """walrus accepts only one sync wait per instruction (and none at all on
    gpsimd DIRECT2D DMAs); hoist excess waits onto standalone EventSemaphore
    instructions inserted before the offender (same engine => same sequencer
    order)."""
    for fn in nc.m.functions:
        for bb in fn.blocks:
            new = []
            changed = False
            for inst in bb.instructions:
                si = inst.sync_info
                keep = 0 if (
                    si is not None
                    and inst.engine == mybir.EngineType.Pool
                    and type(inst).__name__ in ("InstDMACopy", "InstTensorCopy")
                    and len(si.on_wait) >= 1
                ) else 1
                if si is not None and len(si.on_wait) > keep:
                    waits = list(si.on_wait)
                    for w in waits[: len(waits) - keep]:
                        ev = mybir.InstEventSemaphore(
                            name=nc.get_next_instruction_name()
                        )
                        ev.engine = inst.engine
                        ev.sync_info = _br.SyncInfo(on_wait=[w], on_update=[])
                        nc.register_instruction(ev)
                        new.append(ev)
                    inst.sync_info = _br.SyncInfo(
                        on_wait=waits[len(waits) - keep:],
                        on_update=list(si.on_update),
                    )
                    changed = True
                new.append(inst)
            if changed:
                bb.instructions = new


def _build():
    nc = bass.Bass(trn_type="TRN2", target_bir_lowering=False, debug=False)
    x = nc.dram_tensor("x", [E_PER, CAP, D], F32, kind="ExternalInput").ap()
    w_fc = nc.dram_tensor("w_fc", [E_PER, D, H], F32, kind="ExternalInput").ap()
    w_proj = nc.dram_tensor("w_proj", [E_PER, H, D], F32, kind="ExternalInput").ap()
    out = nc.dram_tensor("out", [E_PER, CAP, D], F32, kind="ExternalOutput").ap()
    xbf = [
        [nc.dram_tensor(f"xbf{e}_{s}", [SLAB, D], BF16).ap() for s in range(N_SLABS)]
        for e in range(E_PER)
    ]

    with tile.TileContext(nc) as tc, ExitStack() as ctx:
        xrp = ctx.enter_context(tc.tile_pool(name="xrp", bufs=2))
        xbp = ctx.enter_context(tc.tile_pool(name="xbp", bufs=2))
        xtp = ctx.enter_context(tc.tile_pool(name="xtp", bufs=2 * N_SLABS * KD))
        wload = ctx.enter_context(tc.tile_pool(name="wload", bufs=2))
        wfc_p = ctx.enter_context(tc.tile_pool(name="wfc", bufs=2))
        wproj_p = ctx.enter_context(tc.tile_pool(name="wproj", bufs=2))
        ht_p = ctx.enter_context(tc.tile_pool(name="ht", bufs=8))
        out_p = ctx.enter_context(tc.tile_pool(name="outp", bufs=3))
        ps_h = ctx.enter_context(tc.tile_pool(name="ps_h", bufs=2, space="PSUM"))
        ps_o = ctx.enter_context(tc.tile_pool(name="ps_o", bufs=4, space="PSUM"))

        HPACK = 2          # h_tiles packed per PSUM tile / GELU call

        def load_weights(e):
            wfc_raw = wload.tile([P, KD, H], F32, tag="wl")
            nc.sync.dma_start(wfc_raw[:], w_fc[e].rearrange("(k p) h -> p k h", p=P))
            wfc = wfc_p.tile([P, KD, H], BF16, tag="wfc")
            nc.vector.tensor_copy(wfc[:], wfc_raw[:])
            wproj_raw = wload.tile([P, KH, D], F32, tag="wl")
            nc.sync.dma_start(
                wproj_raw[:], w_proj[e].rearrange("(k p) d -> p k d", p=P)
            )
            wproj = wproj_p.tile([P, KH, D], BF16, tag="wproj")
            nc.vector.tensor_copy(wproj[:], wproj_raw[:])
            return wfc, wproj

        def make_xt(e):
            return [
                [
                    xtp.tile([P, SLAB], BF16, tag="xt", name=f"xt{e}_{k}_{s}")
                    for s in range(N_SLABS)
                ]
                for k in range(KD)
            ]

        xbfs = {}

        def stage_slab(e, s):
            """Load slab s of x[e] into SBUF (f32), DVE-cast to bf16, and
            write the bf16 slab back to its DRAM staging tensor."""
            cs = slice(s * SLAB, (s + 1) * SLAB)
            xraw = xrp.tile([P, Q, D], F32, tag="xr", name=f"xr{e}_{s}")
            nc.sync.dma_start(
                xraw[:], x[e][cs].rearrange("(q p) d -> p q d", p=P)
            )
            xb = xbp.tile([P, Q, D], BF16, tag="xb", name=f"xb{e}_{s}")
            nc.vector.tensor_copy(xb[:], xraw[:])
            nc.sync.dma_start(
                xbf[e][s].rearrange("(q p) d -> p q d", p=P), xb[:]
            )

        def issue_transposes(e, xt, s, half):
            """XBar-transpose one [1024,128] block of the staged bf16 slab
            into its xT tile."""
            nc.sync.dma_start_transpose(
                xt[half][s][:], xbf[e][s][:, half * P:(half + 1) * P]
            )

        # ---- prologue: expert 0's weights + full xT ----
        w = [None] * E_PER
        xts = [None] * E_PER
        w[0] = load_weights(0)
        xts[0] = make_xt(0)
        for s in range(N_SLABS):
            stage_slab(0, s)
            for half in range(KD):
                issue_transposes(0, xts[0], s, half)

        for e in range(E_PER):
            xt = xts[e]
            wfc, wproj = w[e]

            for nci in range(N_CHUNKS):
                # stage expert e+1's xT half a slab per chunk, weights at
                # the first chunk.
                if e + 1 < E_PER:
                    if nci == 0:
                        w[e + 1] = load_weights(e + 1)
                        xts[e + 1] = make_xt(e + 1)
                    if nci % 2 == 0:
                        stage_slab(e + 1, nci // 2)
                    issue_transposes(e + 1, xts[e + 1], nci // 2, nci % 2)


                csl = slice(nci * NC_CHUNK, (nci + 1) * NC_CHUNK)
                sidx = (nci * NC_CHUNK) // SLAB
                soff = (nci * NC_CHUNK) % SLAB
                # ---- MM1 + GELU: HPACK h_tiles per PSUM tile / GELU call ----
                ht_tiles = []
                for hp in range(H_TILES // HPACK):
                    psh = ps_h.tile([P, HPACK, NC_CHUNK], F32, tag="psh")
                    for j in range(HPACK):
                        hi = hp * HPACK + j
                        for k in range(KD):
                            nc.tensor.matmul(
                                psh[:, j, :],
                                wfc[:, k, hi * P:(hi + 1) * P],
                                xt[k][sidx][:, soff:soff + NC_CHUNK],
                                start=(k == 0),
                                stop=(k == KD - 1),
                            )
                    ht = ht_p.tile([P, HPACK, NC_CHUNK], BF16, tag="ht")
                    nc.scalar.activation(
                        ht[:], psh[:], mybir.ActivationFunctionType.Gelu
                    )
                    ht_tiles.append(ht)

                # ---- MM2, k-major: psum[s] += hT[k][s].T @ w_proj[k] ----
                NS = NC_CHUNK // P  # 4 capacity subtiles
                psos = [
                    ps_o.tile([P, 2 * D], F32, tag="pso", name=f"pso{e}_{nci}_{i}")
                    for i in range(NS)
                ]
                for k in range(KH):
                    for s in range(NS):
                        nc.tensor.matmul(
                            psos[s][:, :D],
                            ht_tiles[k // HPACK][:, k % HPACK, s * P:(s + 1) * P],
                            wproj[:, k, :],
                            start=(k == 0),
                            stop=(k == KH - 1),
                        )
                ob = out_p.tile([P, NC_CHUNK // P, D], F32, tag="ob")
                for s, pso in enumerate(psos):
                    nc.vector.tensor_copy(ob[:, s, :], pso[:, :D])
                nc.sync.dma_start(
                    out[e, csl, :].rearrange("(s p) d -> p s d", p=P), ob[:]
                )

    _fix_waits(nc)
    return nc


_CACHE = {}


def _get_nc():
    if "nc" not in _CACHE:
        _CACHE["nc"] = _build()
    return _CACHE["nc"]


def kernel(x, w_fc, w_proj, trace=False):
    assert x.shape == (E, CAP, D) and w_fc.shape == (E, D, H)
    assert w_proj.shape == (E, H, D)
    nc = _get_nc()
    x = np.ascontiguousarray(x, dtype=np.float32)
    w_fc = np.ascontiguousarray(w_fc, dtype=np.float32)
    w_proj = np.ascontiguousarray(w_proj, dtype=np.float32)
    in_maps = [
        {
            "x": x[i * E_PER:(i + 1) * E_PER],
            "w_fc": w_fc[i * E_PER:(i + 1) * E_PER],
            "w_proj": w_proj[i * E_PER:(i + 1) * E_PER],
        }
        for i in range(N_CORES)
    ]
    res = run_bass_kernel_spmd(nc, in_maps, list(range(N_CORES)), trace=trace)
    out = np.concatenate([r["out"] for r in res.results], axis=0)
    if trace:
        kernel.last_results = res
    return out


# revision 17
# speedup vs baseline: 1.7256x; 1.0182x over previous
"""Expert-parallel MoE MLP (ExpertMLP) Bass kernel for 8 Trainium2 NeuronCores.

Problem: x[32,4096,256] @ w_fc[32,256,1024] -> gelu(erf) -> @ w_proj[32,1024,256].

Sharding: expert-parallel. Each of the 8 cores gets 4 experts (slices of the
leading axis of every tensor); no cross-core communication. Inside a core, per
expert e:

  1. xT [d, c] in bf16: each 1024-row slab of x[e] is DMA'd f32 DRAM->SBUF
     in natural [cap, d] orientation (contiguous, fast), DVE-cast to bf16
     (round-to-nearest), written back to a per-slab DRAM staging tensor, and
     XBar DMA-transposed DRAM->SBUF as two [1024,128] blocks. All four steps
     are HWDGE/DVE ops with per-slab dependencies, so the pipeline starts
     ~10us in - unlike the software-DGE DRAM->DRAM cast pass (55us of queue
     drain before the first matmul) or per-[128,128]-block SBUF XBar calls
     (a DMA_TRANSPOSE costs ~1.2us of engine time regardless of size).
  2. MM1: hT[h_tile, c_chunk] += w_fc_tile.T @ xT_chunk - w_fc's natural
     [d, h] layout is the stationary operand (DVE-cast to bf16 on load).
  3. GELU (exact erf form) runs on the ACT engine as the PSUM->SBUF eviction.
  4. MM2 runs k-major over all capacity subtiles of the chunk: for k, for s:
     out[s] += hT[k][s].T @ w_proj[k]. The first MM2 instructions only need
     the chunk's first GELU group, so the PE never waits for the last GELU.
     Each subtile accumulator owns a FULL 2KB PSUM bank (start=True clears
     the whole bank). Results land directly in [capacity, d] orientation.

xT staging for expert e+1 is issued interleaved with expert e's chunk loop
(half a slab per chunk) so the SP queue serves output DMAs promptly and xT
production stays ~1 expert ahead of consumption.

All matmul operands are bf16; PSUM accumulation stays fp32.
"""

import numpy as np
from contextlib import ExitStack

import bass_rust as _br
import concourse.bass as bass
import concourse.tile as tile
from concourse import mybir
from concourse.bass_utils import run_bass_kernel_spmd

E, CAP, D, H = 32, 4096, 256, 1024
N_CORES = 8
E_PER = E // N_CORES  # 4 experts per core
P = 128
F32 = mybir.dt.float32
BF16 = mybir.dt.bfloat16

KD = D // P        # 2 k-tiles in MM1's contraction
KH = H // P        # 8 k-tiles in MM2's contraction
NC_CHUNK = 512     # capacity chunk processed per MM1/MM2 round
N_CHUNKS = CAP // NC_CHUNK
H_TILES = H // P
SLAB = 1024        # xT staging slab (capacity rows)
N_SLABS = CAP // SLAB
Q = SLAB // P      # 8 capacity subtiles per slab


def _fix_waits(nc):
    """walrus accepts only one sync wait per instruction (and none at all on
    gpsimd DIRECT2D DMAs); hoist excess waits onto standalone EventSemaphore
    instructions inserted before the offender (same engine => same sequencer
    order)."""
    for fn in nc.m.functions:
        for bb in fn.blocks:
            new = []
            changed = False
            for inst in bb.instructions:
                si = inst.sync_info
                keep = 0 if (
                    si is not None
                    and inst.engine == mybir.EngineType.Pool
                    and type(inst).__name__ in ("InstDMACopy", "InstTensorCopy")
                    and len(si.on_wait) >= 1
                ) else 1
                if si is not None and len(si.on_wait) > keep:
                    waits = list(si.on_wait)
                    for w in waits[: len(waits) - keep]:
                        ev = mybir.InstEventSemaphore(
                            name=nc.get_next_instruction_name()
                        )
                        ev.engine = inst.engine
                        ev.sync_info = _br.SyncInfo(on_wait=[w], on_update=[])
                        nc.register_instruction(ev)
                        new.append(ev)
                    inst.sync_info = _br.SyncInfo(
                        on_wait=waits[len(waits) - keep:],
                        on_update=list(si.on_update),
                    )
                    changed = True
                new.append(inst)
            if changed:
                bb.instructions = new


def _build():
    nc = bass.Bass(trn_type="TRN2", target_bir_lowering=False, debug=False)
    x = nc.dram_tensor("x", [E_PER, CAP, D], F32, kind="ExternalInput").ap()
    w_fc = nc.dram_tensor("w_fc", [E_PER, D, H], F32, kind="ExternalInput").ap()
    w_proj = nc.dram_tensor("w_proj", [E_PER, H, D], F32, kind="ExternalInput").ap()
    out = nc.dram_tensor("out", [E_PER, CAP, D], F32, kind="ExternalOutput").ap()
    xbf = [
        [nc.dram_tensor(f"xbf{e}_{s}", [SLAB, D], BF16).ap() for s in range(N_SLABS)]
        for e in range(E_PER)
    ]

    with tile.TileContext(nc) as tc, ExitStack() as ctx:
        xrp = ctx.enter_context(tc.tile_pool(name="xrp", bufs=2))
        xbp = ctx.enter_context(tc.tile_pool(name="xbp", bufs=2))
        xtp = ctx.enter_context(tc.tile_pool(name="xtp", bufs=2 * N_SLABS * KD))
        wload = ctx.enter_context(tc.tile_pool(name="wload", bufs=2))
        wfc_p = ctx.enter_context(tc.tile_pool(name="wfc", bufs=2))
        wproj_p = ctx.enter_context(tc.tile_pool(name="wproj", bufs=2))
        ht_p = ctx.enter_context(tc.tile_pool(name="ht", bufs=8))
        out_p = ctx.enter_context(tc.tile_pool(name="outp", bufs=3))
        ps_h = ctx.enter_context(tc.tile_pool(name="ps_h", bufs=2, space="PSUM"))
        ps_o = ctx.enter_context(tc.tile_pool(name="ps_o", bufs=4, space="PSUM"))

        HPACK = 2          # h_tiles packed per PSUM tile / GELU call

        def load_weights(e):
            wfc_raw = wload.tile([P, KD, H], F32, tag="wl")
            nc.sync.dma_start(wfc_raw[:], w_fc[e].rearrange("(k p) h -> p k h", p=P))
            wfc = wfc_p.tile([P, KD, H], BF16, tag="wfc")
            nc.vector.tensor_copy(wfc[:], wfc_raw[:])
            wproj_raw = wload.tile([P, KH, D], F32, tag="wl")
            nc.sync.dma_start(
                wproj_raw[:], w_proj[e].rearrange("(k p) d -> p k d", p=P)
            )
            wproj = wproj_p.tile([P, KH, D], BF16, tag="wproj")
            nc.vector.tensor_copy(wproj[:], wproj_raw[:])
            return wfc, wproj

        def make_xt(e):
            return [
                [
                    xtp.tile([P, SLAB], BF16, tag="xt", name=f"xt{e}_{k}_{s}")
                    for s in range(N_SLABS)
                ]
                for k in range(KD)
            ]

        xbfs = {}

        def stage_slab(e, s):
            """Load slab s of x[e] into SBUF (f32), DVE-cast to bf16, and
            write the bf16 slab back to its DRAM staging tensor."""
            cs = slice(s * SLAB, (s + 1) * SLAB)
            xraw = xrp.tile([P, Q, D], F32, tag="xr", name=f"xr{e}_{s}")
            nc.sync.dma_start(
                xraw[:], x[e][cs].rearrange("(p q) d -> p q d", p=P)
            )
            xb = xbp.tile([P, Q, D], BF16, tag="xb", name=f"xb{e}_{s}")
            nc.vector.tensor_copy(xb[:], xraw[:])
            nc.sync.dma_start(
                xbf[e][s].rearrange("(p q) d -> p q d", p=P), xb[:]
            )

        def issue_transposes(e, xt, s, half):
            """XBar-transpose one [1024,128] block of the staged bf16 slab
            into its xT tile."""
            nc.sync.dma_start_transpose(
                xt[half][s][:], xbf[e][s][:, half * P:(half + 1) * P]
            )

        # ---- prologue: expert 0's weights + full xT ----
        w = [None] * E_PER
        xts = [None] * E_PER
        w[0] = load_weights(0)
        xts[0] = make_xt(0)
        for s in range(N_SLABS):
            stage_slab(0, s)
            for half in range(KD):
                issue_transposes(0, xts[0], s, half)

        for e in range(E_PER):
            xt = xts[e]
            wfc, wproj = w[e]

            for nci in range(N_CHUNKS):
                # stage expert e+1's xT half a slab per chunk, weights at
                # the first chunk.
                if e + 1 < E_PER:
                    if nci == 0:
                        w[e + 1] = load_weights(e + 1)
                        xts[e + 1] = make_xt(e + 1)
                    if nci % 2 == 0:
                        stage_slab(e + 1, nci // 2)
                    issue_transposes(e + 1, xts[e + 1], nci // 2, nci % 2)


                csl = slice(nci * NC_CHUNK, (nci + 1) * NC_CHUNK)
                sidx = (nci * NC_CHUNK) // SLAB
                soff = (nci * NC_CHUNK) % SLAB
                # ---- MM1 + GELU: HPACK h_tiles per PSUM tile / GELU call ----
                ht_tiles = []
                for hp in range(H_TILES // HPACK):
                    psh = ps_h.tile([P, HPACK, NC_CHUNK], F32, tag="psh")
                    for j in range(HPACK):
                        hi = hp * HPACK + j
                        for k in range(KD):
                            nc.tensor.matmul(
                                psh[:, j, :],
                                wfc[:, k, hi * P:(hi + 1) * P],
                                xt[k][sidx][:, soff:soff + NC_CHUNK],
                                start=(k == 0),
                                stop=(k == KD - 1),
                            )
                    ht = ht_p.tile([P, HPACK, NC_CHUNK], BF16, tag="ht")
                    nc.scalar.activation(
                        ht[:], psh[:], mybir.ActivationFunctionType.Gelu
                    )
                    ht_tiles.append(ht)

                # ---- MM2, k-major: psum[s] += hT[k][s].T @ w_proj[k] ----
                NS = NC_CHUNK // P  # 4 capacity subtiles
                psos = [
                    ps_o.tile([P, 2 * D], F32, tag="pso", name=f"pso{e}_{nci}_{i}")
                    for i in range(NS)
                ]
                for k in range(KH):
                    for s in range(NS):
                        nc.tensor.matmul(
                            psos[s][:, :D],
                            ht_tiles[k // HPACK][:, k % HPACK, s * P:(s + 1) * P],
                            wproj[:, k, :],
                            start=(k == 0),
                            stop=(k == KH - 1),
                        )
                ob = out_p.tile([P, NC_CHUNK // P, D], F32, tag="ob")
                for s, pso in enumerate(psos):
                    nc.vector.tensor_copy(ob[:, s, :], pso[:, :D])
                nc.sync.dma_start(
                    out[e, csl, :].rearrange("(s p) d -> p s d", p=P), ob[:]
                )

    _fix_waits(nc)
    return nc


_CACHE = {}


def _get_nc():
    if "nc" not in _CACHE:
        _CACHE["nc"] = _build()
    return _CACHE["nc"]


def kernel(x, w_fc, w_proj, trace=False):
    assert x.shape == (E, CAP, D) and w_fc.shape == (E, D, H)
    assert w_proj.shape == (E, H, D)
    nc = _get_nc()
    x = np.ascontiguousarray(x, dtype=np.float32)
    w_fc = np.ascontiguousarray(w_fc, dtype=np.float32)
    w_proj = np.ascontiguousarray(w_proj, dtype=np.float32)
    in_maps = [
        {
            "x": x[i * E_PER:(i + 1) * E_PER],
            "w_fc": w_fc[i * E_PER:(i + 1) * E_PER],
            "w_proj": w_proj[i * E_PER:(i + 1) * E_PER],
        }
        for i in range(N_CORES)
    ]
    res = run_bass_kernel_spmd(nc, in_maps, list(range(N_CORES)), trace=trace)
    out = np.concatenate([r["out"] for r in res.results], axis=0)
    if trace:
        kernel.last_results = res
    return out


# revision 18
# speedup vs baseline: 2.0608x; 1.1943x over previous
"""Expert-parallel MoE MLP (ExpertMLP) Bass kernel for 8 Trainium2 NeuronCores.

Problem: x[32,4096,256] @ w_fc[32,256,1024] -> gelu(erf) -> @ w_proj[32,1024,256].

Sharding: expert-parallel. Each of the 8 cores gets 4 experts (slices of the
leading axis of every tensor); no cross-core communication. Inside a core, per
expert e:

  1. x[e] ([4096,256], capacity-major) is transposed on the PE (identity
     matmul, 128x128 blocks) into xT [d, c] so the d-contraction of the first
     matmul lies on the partition axis.
  2. MM1: hT[h_tile, c_chunk] += w_fc_tile.T @ xT_chunk - w_fc's natural
     [d, h] layout is the stationary operand, so it needs no transpose.
  3. GELU (exact erf form) runs on the ACT engine as the PSUM->SBUF eviction.
  4. MM2 uses hT slices as the *stationary* operand and w_proj's natural
     [h, d] layout as the moving operand: out[c_sub, d] += hT_slice.T @
     w_proj_tile. The result lands directly in [capacity, d] orientation, so
     no output transpose is needed.

All matmul operands are float32r (e8m11, 1 PE cycle/row at N>=256 vs 4 for
fp32); producers (DVE copies / ACT gelu) write f32r tiles, which performs the
required rounding. PSUM accumulation stays fp32.
"""

import numpy as np
from contextlib import ExitStack

import bass_rust as _br
import concourse.bass as bass
import concourse.tile as tile
from concourse import mybir
from concourse.bass_utils import run_bass_kernel_spmd
from concourse.masks import make_identity

E, CAP, D, H = 32, 4096, 256, 1024
N_CORES = 8
E_PER = E // N_CORES  # 4 experts per core
P = 128
F32 = mybir.dt.float32
F32R = mybir.dt.float32r
BF16 = mybir.dt.bfloat16

KD = D // P        # 2 k-tiles in MM1's contraction
KH = H // P        # 8 k-tiles in MM2's contraction
NC_CHUNK = 512     # capacity chunk processed per MM1/MM2 round
N_CHUNKS = CAP // NC_CHUNK
H_TILES = H // P
C_TILES = CAP // P


def _fix_waits(nc):
    """walrus here accepts only one sync wait per instruction; hoist excess
    waits onto standalone EventSemaphore instructions inserted before the
    offender (same engine => same sequencer order)."""
    for fn in nc.m.functions:
        for bb in fn.blocks:
            new = []
            changed = False
            for inst in bb.instructions:
                si = inst.sync_info
                if si is not None and len(si.on_wait) > 1:
                    waits = list(si.on_wait)
                    for w in waits[:-1]:
                        ev = mybir.InstEventSemaphore(
                            name=nc.get_next_instruction_name()
                        )
                        ev.engine = inst.engine
                        ev.sync_info = _br.SyncInfo(on_wait=[w], on_update=[])
                        nc.register_instruction(ev)
                        new.append(ev)
                    inst.sync_info = _br.SyncInfo(
                        on_wait=waits[-1:], on_update=list(si.on_update)
                    )
                    changed = True
                new.append(inst)
            if changed:
                bb.instructions = new


def _build():
    nc = bass.Bass(trn_type="TRN2", target_bir_lowering=False, debug=False)
    x = nc.dram_tensor("x", [E_PER, CAP, D], F32, kind="ExternalInput").ap()
    w_fc = nc.dram_tensor("w_fc", [E_PER, D, H], F32, kind="ExternalInput").ap()
    w_proj = nc.dram_tensor("w_proj", [E_PER, H, D], F32, kind="ExternalInput").ap()
    out = nc.dram_tensor("out", [E_PER, CAP, D], F32, kind="ExternalOutput").ap()
    # bf16 staging copies of x so the XBar DMA-transpose (2-byte dtype only)
    # can build xT without burning TensorE cycles on identity transposes.
    # One DRAM tensor per (expert, half): DRAM dependency tracking is
    # tensor-granular, so finer tensors let each transpose start as soon as
    # its own cast chunk lands instead of after all casts.
    CASTCH = CAP // 2  # cast-DMA chunk (rows)
    xbf = [
        [
            nc.dram_tensor(f"xbf{e}_{hh}", [CASTCH, D], BF16).ap()
            for hh in range(CAP // CASTCH)
        ]
        for e in range(E_PER)
    ]

    with tile.TileContext(nc) as tc, ExitStack() as ctx:
        xtp = ctx.enter_context(tc.tile_pool(name="xtp", bufs=2 * E_PER))
        wload = ctx.enter_context(tc.tile_pool(name="wload", bufs=2))
        wfc_p = ctx.enter_context(tc.tile_pool(name="wfc", bufs=2))
        wproj_p = ctx.enter_context(tc.tile_pool(name="wproj", bufs=2))
        ht_p = ctx.enter_context(tc.tile_pool(name="ht", bufs=8))
        out_p = ctx.enter_context(tc.tile_pool(name="outp", bufs=3))
        ps_h = ctx.enter_context(tc.tile_pool(name="ps_h", bufs=2, space="PSUM"))
        ps_o = ctx.enter_context(tc.tile_pool(name="ps_o", bufs=4, space="PSUM"))

        HPACK = 2          # h_tiles packed per PSUM tile / GELU call
        SLAB = 1024        # DMA-transpose slab (capacity columns)

        def load_weights(e):
            wfc_raw = wload.tile([P, KD, H], F32, tag="wl")
            nc.sync.dma_start(wfc_raw[:], w_fc[e].rearrange("(k p) h -> p k h", p=P))
            wfc = wfc_p.tile([P, KD, H], BF16, tag="wfc")
            nc.vector.tensor_copy(wfc[:], wfc_raw[:])
            wproj_raw = wload.tile([P, KH, D], F32, tag="wl")
            nc.sync.dma_start(
                wproj_raw[:], w_proj[e].rearrange("(k p) d -> p k d", p=P)
            )
            wproj = wproj_p.tile([P, KH, D], BF16, tag="wproj")
            nc.vector.tensor_copy(wproj[:], wproj_raw[:])
            return wfc, wproj

        # ---- prologue: expert 0's weights first, then stage all experts' xT:
        # DRAM->DRAM cast x[e]->bf16 in half-chunks (q0 FIFO => e0 first),
        # then XBar-transpose 1024-column slabs into SBUF on the scalar HWDGE
        # queue so they don't queue behind weight/output traffic on q1.
        # MM1 of (e, chunk) only needs its slab, so compute starts early.
        w0 = load_weights(0)
        for e in range(E_PER):
            for hh in range(CAP // CASTCH):
                rs = slice(hh * CASTCH, (hh + 1) * CASTCH)
                nc.gpsimd.dma_start(xbf[e][hh][:], x[e][rs])
        SPH = CASTCH // SLAB  # slabs per cast half
        xts = []
        for e in range(E_PER):
            xt = [
                [
                    xtp.tile([P, SLAB], BF16, tag="xt", name=f"xt{e}_{k}_{s}")
                    for s in range(CAP // SLAB)
                ]
                for k in range(KD)
            ]
            for s in range(CAP // SLAB):
                ls = slice((s % SPH) * SLAB, (s % SPH + 1) * SLAB)
                for k in range(KD):
                    nc.sync.dma_start_transpose(
                        xt[k][s][:], xbf[e][s // SPH][ls, k * P:(k + 1) * P]
                    )
            xts.append(xt)

        for e in range(E_PER):
            xt = xts[e]
            wfc, wproj = w0 if e == 0 else load_weights(e)

            # ---- MM1 -> GELU -> MM2 per capacity chunk ----
            # MM1 accumulates HPACK h_tiles into one multi-bank PSUM tile so
            # GELU evicts in wider (cheaper) ACTIVATE calls; hT is written in
            # bf16 so MM2's per-matmul weight loads run at 2-byte FWL speed.
            for nci in range(N_CHUNKS):
                csl = slice(nci * NC_CHUNK, (nci + 1) * NC_CHUNK)
                ht_tiles = []  # HPACK-wide bf16 tiles
                for hp in range(H_TILES // HPACK):
                    psh = ps_h.tile([P, HPACK, NC_CHUNK], F32, tag="psh")
                    for j in range(HPACK):
                        hi = hp * HPACK + j
                        for k in range(KD):
                            sidx = (nci * NC_CHUNK) // SLAB
                            soff = (nci * NC_CHUNK) % SLAB
                            nc.tensor.matmul(
                                psh[:, j, :],
                                wfc[:, k, hi * P:(hi + 1) * P],
                                xt[k][sidx][:, soff:soff + NC_CHUNK],
                                start=(k == 0),
                                stop=(k == KD - 1),
                            )
                    ht = ht_p.tile([P, HPACK, NC_CHUNK], BF16, tag="ht")
                    nc.scalar.activation(
                        ht[:], psh[:], mybir.ActivationFunctionType.Gelu
                    )
                    ht_tiles.append(ht)

                ob = out_p.tile([P, NC_CHUNK // P, D], F32, tag="ob")
                for s in range(NC_CHUNK // P):
                    pso = ps_o.tile([P, D], F32, tag="pso")
                    for k in range(KH):
                        nc.tensor.matmul(
                            pso[:],
                            ht_tiles[k // HPACK][:, k % HPACK, s * P:(s + 1) * P],
                            wproj[:, k, :],
                            start=(k == 0),
                            stop=(k == KH - 1),
                        )
                    nc.vector.tensor_copy(ob[:, s, :], pso[:])
                nc.sync.dma_start(
                    out[e, csl, :].rearrange("(s p) d -> p s d", p=P), ob[:]
                )

    _fix_waits(nc)
    return nc


_CACHE = {}


def _get_nc():
    if "nc" not in _CACHE:
        _CACHE["nc"] = _build()
    return _CACHE["nc"]


def kernel(x, w_fc, w_proj, trace=False):
    assert x.shape == (E, CAP, D) and w_fc.shape == (E, D, H)
    assert w_proj.shape == (E, H, D)
    nc = _get_nc()
    x = np.ascontiguousarray(x, dtype=np.float32)
    w_fc = np.ascontiguousarray(w_fc, dtype=np.float32)
    w_proj = np.ascontiguousarray(w_proj, dtype=np.float32)
    in_maps = [
        {
            "x": x[i * E_PER:(i + 1) * E_PER],
            "w_fc": w_fc[i * E_PER:(i + 1) * E_PER],
            "w_proj": w_proj[i * E_PER:(i + 1) * E_PER],
        }
        for i in range(N_CORES)
    ]
    res = run_bass_kernel_spmd(nc, in_maps, list(range(N_CORES)), trace=trace)
    out = np.concatenate([r["out"] for r in res.results], axis=0)
    if trace:
        kernel.last_results = res
    return out

